# revision 10
# baseline (speedup 1.0000x reference)
"""ChildSumTreeLSTM (N=8192 complete 8-ary tree) on 8 TRN2 NeuronCores.

Decomposition (all tree structure is compile-time static):
- nodes 0..1023 are internal (children of p = 8p+1..8p+8), 1024..8191 leaves.
- Phase A (per core): iou_x/fx_x projections for the ~1096 node-columns this
  core owns, feature-major, bf16 matmuls on the PE.
- Leaf phase: elementwise sigmoid/tanh -> leaf (h, c), kept resident in SBUF.
- 5 sequential rounds of internal levels: R4 (parents 585..1023, 439),
  R3 (73..584, 512), R2 (9..72, 64), R1 (1..8, 8), R0 (root).
  Each round is node-sharded across the 8 cores so that every child a core
  needs was computed locally, except: R4 results are AllGather'ed (core 0
  consumes them for R3), and R1 results are AllGather'ed (every core then
  computes the root; core 0's answer is returned).

Perf notes vs the f32 version: all matmul operands are bf16 (halves weight
DMA, enables fast-weight-load), the recurrent weight set loads concurrently
with Phase A on a separate DMA queue, leaf (h, c) never leaves SBUF, and the
AllGather payloads are half-width.
"""
import sys
import functools

sys.path.insert(0, '/opt/trn_rl_repo')

import numpy as np
import concourse.bacc as bacc
import concourse.mybir as mybir
import concourse.tile as tile
from concourse.bass_utils import run_bass_kernel_spmd

DT = mybir.dt
AF = mybir.ActivationFunctionType

NCORES = 8
N = 8192
M = 1024
C4 = [54, 54, 55, 55, 55, 55, 55, 56]
S4 = [585, 639, 693, 748, 803, 858, 913, 968]
NB = [56, 64, 8, 1, 1]          # parents per round (uniform per core)
Q0 = [0, 56, 120, 128, 129]     # row offset into the node-major parent table
NCOLS = 1096                    # 448 (R4 children) + 512 (R3 children) + 136 parents
SBW = 548                       # superblock width (2 superblocks)
HBW = 274                       # half-block (matmul moving operand <= 512)


def _core_cols(i):
    cols = []
    for pl in range(56):
        for k in range(8):
            if pl < C4[i]:
                node = 8 * (S4[i] + pl) + 1 + k
                cols.append(node if node < N else -1)
            else:
                cols.append(-1)
    for b in range(512):
        node = 585 + 512 * i + b
        cols.append(-1 if (i == 0 and b < 439) else node)
    for q in range(56):
        cols.append(S4[i] + q if q < C4[i] else -1)
    cols += [73 + 64 * i + j for j in range(64)]
    cols += [9 + 8 * i + j for j in range(8)]
    cols += [1 + i, 0] + [-1] * 6
    return cols


@functools.lru_cache(maxsize=1)
def _build():
    nc = bacc.Bacc(trn_type="TRN2", target_bir_lowering=False, debug=False,
                   num_devices=NCORES)

    BF = DT.bfloat16
    xT_d = nc.dram_tensor("xT", [8, 128, NCOLS], BF, kind="ExternalInput")
    WA_d = nc.dram_tensor("WA", [8, 128, 4096], BF, kind="ExternalInput")
    WR_d = nc.dram_tensor("WR", [8, 128, 4096], BF, kind="ExternalInput")
    BT_d = nc.dram_tensor("BT", [128, 32], DT.float32, kind="ExternalInput")
    SEL_d = nc.dram_tensor("SEL", [128, 1024], BF, kind="ExternalInput")
    I_d = nc.dram_tensor("I128", [128, 128], BF, kind="ExternalInput")
    rh_d = nc.dram_tensor("root_h", [1, M], DT.float32, kind="ExternalOutput")
    rc_d = nc.dram_tensor("root_c", [1, M], DT.float32, kind="ExternalOutput")

    RG = [list(range(NCORES))]

    with tile.TileContext(nc) as tc:
        with (
            tc.tile_pool(name="dram", bufs=1, space="DRAM") as dram,
            tc.tile_pool(name="persist", bufs=1) as pp,
            tc.tile_pool(name="wpool", bufs=1) as wp,
        ):
            nm_dram = dram.tile([136, 1024], BF)
            ag_in = dram.tile([128, 2, 8, 56], BF)
            ag_out = dram.tile([NCORES, 128, 2, 8, 56], BF,
                               addr_space="Shared")
            agb_in = dram.tile([128, 2, 8, 1], BF)
            agb_out = dram.tile([NCORES, 128, 2, 8, 1], BF,
                                addr_space="Shared")

            I_t = pp.tile([128, 128], BF)
            SEL_t = pp.tile([128, 1024], BF)
            BT_t = pp.tile([128, 32], DT.float32)
            nc.scalar.dma_start(I_t[:], I_d[:])
            nc.scalar.dma_start(SEL_t[:], SEL_d[:])
            nc.scalar.dma_start(BT_t[:], BT_d[:])

            # leaf/child (h, c), feature-major, SBUF-resident:
            # hc[p, state, m, col] with state 0 = c, 1 = h
            hc = pp.tile([128, 2, 8, 888], BF)
            hcB = pp.tile([128, 2, 8, 72], BF)
            # feature-major iou_x (+bias) for the 136 parent columns
            iouxf = pp.tile([128, 24, 136], BF)

            # x for both superblocks loads first on the scalar DGE; WA
            # follows on the same queue. Neither sits behind the sync-queue
            # startup barrier, so Phase A starts ~45us earlier.
            xs2 = pp.tile([128, 2, 8, SBW], BF)
            for sb in range(2):
                nc.scalar.dma_start(
                    xs2[:, sb],
                    xT_d[:, :, sb * SBW:(sb + 1) * SBW].rearrange(
                        "k p w -> p k w"))
            WA_t = wp.tile([128, 8, 4096], BF, tag="wa", bufs=1)
            WR_t = wp.tile([128, 8, 4096], BF, tag="wr", bufs=1)
            for k in range(8):
                nc.scalar.dma_start(WA_t[:, k, :], WA_d[k])
            # recurrent weights stream on the gpsimd DMA queue, overlapping
            # Phase A's compute instead of serializing after it
            for k in range(8):
                nc.gpsimd.dma_start(WR_t[:, k, :], WR_d[k])

            # ---------------- Phase A + leaves ----------------
            with (
                tc.tile_pool(name="xp", bufs=2) as xp,
                tc.tile_pool(name="pap", bufs=1, space="PSUM") as pap,
                tc.tile_pool(name="drp", bufs=1) as drp,
            ):
                for sb in range(2):
                    xs = xs2[:, sb]
                    ngate = 3 if sb == 0 else 4
                    lw = SBW if sb == 0 else 412  # leaf cols in this superblock
                    for jm in range(8):
                        ps = {}
                        for gi in range(ngate):
                            for b in range(2):
                                if gi == 3 and b == 0:
                                    continue  # f-gate only needed for parents
                                ps[gi, b] = pap.tile([128, HBW], DT.float32,
                                                     tag="pa", bufs=8,
                                                     name=f"pa_{sb}_{jm}_{gi}_{b}")
                        for k in range(8):
                            for gi in range(ngate):
                                j = jm + 8 * gi
                                lhsT = WA_t[:, k, 128 * j:128 * (j + 1)]
                                for b in range(2):
                                    if (gi, b) not in ps:
                                        continue
                                    nc.tensor.matmul(
                                        ps[gi, b][:],
                                        lhsT,
                                        xs[:, k, HBW * b:HBW * (b + 1)],
                                        start=(k == 0), stop=(k == 7),
                                    )
                        # leaf elementwise drains -> SBUF hc planes
                        for b in range(2):
                            bw = min(HBW, max(0, lw - HBW * b))
                            if bw > 0:
                                cb = sb * SBW + HBW * b
                                # split the write at col 888 (hc | hcB)
                                if cb + bw <= 888:
                                    segs = [(hc, cb, 0, bw)]
                                elif cb >= 888:
                                    segs = [(hcB, cb - 888, 0, bw)]
                                else:
                                    segs = [(hc, cb, 0, 888 - cb),
                                            (hcB, 0, 888 - cb, cb + bw - 888)]
                                si = drp.tile([128, HBW], BF, tag="dr",
                                              bufs=8, name=f"si_{sb}_{jm}_{b}")
                                tu = drp.tile([128, HBW], BF, tag="dr",
                                              bufs=8, name=f"tu_{sb}_{jm}_{b}")
                                nc.scalar.activation(si[:, 0:bw], ps[0, b][:, 0:bw],
                                                     AF.Sigmoid,
                                                     bias=BT_t[:, jm:jm + 1])
                                nc.scalar.activation(tu[:, 0:bw], ps[2, b][:, 0:bw],
                                                     AF.Tanh,
                                                     bias=BT_t[:, jm + 16:jm + 17])
                                so = drp.tile([128, HBW], BF, tag="dr",
                                              bufs=8, name=f"so_{sb}_{jm}_{b}")
                                nc.scalar.activation(so[:, 0:bw], ps[1, b][:, 0:bw],
                                                     AF.Sigmoid,
                                                     bias=BT_t[:, jm + 8:jm + 9])
                                tanc = drp.tile([128, HBW], BF, tag="dr",
                                                bufs=8, name=f"tanc_{sb}_{jm}_{b}")
                                for tgt, tc0, s0, sw in segs:
                                    cv = tgt[:, 0, jm, tc0:tc0 + sw]
                                    nc.vector.tensor_mul(cv, si[:, s0:s0 + sw],
                                                         tu[:, s0:s0 + sw])
                                    nc.scalar.activation(tanc[:, s0:s0 + sw], cv,
                                                         AF.Tanh)
                                    nc.vector.tensor_mul(
                                        tgt[:, 1, jm, tc0:tc0 + sw],
                                        so[:, s0:s0 + sw], tanc[:, s0:s0 + sw])
                        # parent drains + transpose to node-major (sb 1, b 1,
                        # cols 138..274 of the half-block)
                        if sb == 1:
                            for gi in range(3):
                                j = jm + 8 * gi
                                nc.scalar.activation(
                                    iouxf[:, j, 0:136], ps[gi, 1][:, 138:274],
                                    AF.Identity, bias=BT_t[:, j:j + 1])
                            fm = drp.tile([128, HBW], BF, tag="dr",
                                          bufs=8, name=f"fm_{jm}")
                            nc.scalar.activation(
                                fm[:, 0:136], ps[3, 1][:, 138:274], AF.Identity,
                                bias=BT_t[:, 24 + jm:25 + jm])
                            for half in range(2):
                                qn = 128 if half == 0 else 8
                                tp = pap.tile([128, HBW], BF,
                                              tag="pa", bufs=8,
                                              name=f"tp_{jm}_{half}")
                                nc.tensor.transpose(
                                    tp[0:qn, 0:128],
                                    fm[:, 128 * half:128 * half + qn],
                                    I_t[:, :])
                                tsb = drp.tile([128, HBW], BF,
                                               tag="dr", bufs=8,
                                               name=f"tsb_{jm}_{half}")
                                nc.vector.tensor_copy(tsb[0:qn, 0:128],
                                                      tp[0:qn, 0:128])
                                nc.sync.dma_start(
                                    nm_dram[128 * half:128 * half + qn,
                                            128 * jm:128 * (jm + 1)],
                                    tsb[0:qn, 0:128])
                # zero the single real pad-child column (local col 447:
                # node 1023's 8th child on core 7; harmless on other cores)
                nc.vector.memset(hc[:, :, :, 447], 0.0)

            # ---------------- Rounds ----------------
            with (
                tc.tile_pool(name="rps", bufs=1, space="PSUM") as rps,
                tc.tile_pool(name="nmp", bufs=1) as nmp,
                tc.tile_pool(name="rwp", bufs=1) as rwp,
                tc.tile_pool(name="sink", bufs=1) as sink,
            ):
                st4 = sink.tile([128, 2, 8, 56], BF)
                c3 = sink.tile([128, 2, 8, 64], BF)
                c2 = sink.tile([128, 2, 8, 8], BF)
                c1 = sink.tile([128, 2, 8, 8], BF)
                st1 = sink.tile([128, 2, 8, 1], BF)
                rootc_sb = sink.tile([128, 8], DT.float32)
                rooth_sb = sink.tile([128, 8], DT.float32)

                def group8_sum(prod_ap, out_ap, nb, rn, jm):
                    """out[p, n] = sum_k prod[p, 8n + k]."""
                    a = prod_ap.rearrange("p (n k) -> p n k", k=8)
                    l1 = rwp.tile([128, 256], BF, tag="lvl1", bufs=2,
                                  name=f"l1_{rn}_{jm}")
                    l1v = l1[:, 0:nb * 4].rearrange("p (n k) -> p n k", k=4)
                    nc.vector.tensor_add(l1v, a[:, :, 0:4], a[:, :, 4:8])
                    l2 = rwp.tile([128, 128], BF, tag="lvl2", bufs=2,
                                  name=f"l2_{rn}_{jm}")
                    l2v = l2[:, 0:nb * 2].rearrange("p (n k) -> p n k", k=2)
                    nc.vector.tensor_add(l2v, l1v[:, :, 0:2], l1v[:, :, 2:4])
                    # out[p, n] = l2[p, 2n] + l2[p, 2n+1]  (stride-2 views)
                    e0 = l2v[:, :, 0:1].rearrange("p n k -> p (n k)")
                    e1 = l2v[:, :, 1:2].rearrange("p n k -> p (n k)")
                    nc.vector.tensor_add(out_ap, e0, e1)

                def run_round(rn, nb, q0, get_chC, get_chH, out_c, out_h):
                    """Feature-major round: psums are [128, 8, nb] per gate."""
                    w8 = 8 * nb
                    # 1. csum (feature-major)
                    csumT = rwp.tile([128, 8, 64], BF, tag="csum",
                                     bufs=2, name=f"csum_{rn}")
                    for m in range(8):
                        group8_sum(get_chC(m), csumT[:, m, 0:nb], nb, rn, m)
                    # 2. iou psums, one [128, 8, nb] bank tile per gate;
                    # x-side (+bias) folded in via an identity matmul from the
                    # feature-major parent table.
                    gps = []
                    for g in range(3):
                        pg = rps.tile([128, 8, NB[rn]], DT.float32, tag="iou",
                                      bufs=3, name=f"ips_{rn}_{g}")
                        for m in range(8):
                            j = m + 8 * g
                            for k in range(8):
                                nc.tensor.matmul(
                                    pg[:, m, 0:nb],
                                    WR_t[:, k, 128 * j:128 * (j + 1)],
                                    csumT[:, k, 0:nb],
                                    start=(k == 0), stop=False)
                            nc.tensor.matmul(
                                pg[:, m, 0:nb],
                                I_t[:, :],
                                iouxf[:, j, q0:q0 + nb],
                                start=False, stop=True)
                        gps.append(pg)
                    # 3-5. f gates (feature-major), prod, fc
                    fxb_t = nmp.tile([64, 1024], BF, tag="nm", bufs=2,
                                     name=f"fxb_{rn}")
                    nc.sync.dma_start(fxb_t[0:nb, :], nm_dram[q0:q0 + nb, :])
                    fcT = rwp.tile([128, 8, 64], BF, tag="fcT", bufs=2,
                                   name=f"fcT_{rn}")
                    for j in range(8):
                        fps = rps.tile([128, 512], DT.float32, tag="fp", bufs=2,
                                       name=f"fps_{rn}_{j}")
                        for k in range(8):
                            nc.tensor.matmul(
                                fps[:, 0:w8],
                                WR_t[:, k, 3072 + 128 * j:3072 + 128 * (j + 1)],
                                get_chC(k)[:, 0:w8],
                                start=(k == 0), stop=False)
                        nc.tensor.matmul(
                            fps[:, 0:w8],
                            fxb_t[0:nb, 128 * j:128 * (j + 1)],
                            SEL_t[0:nb, 0:w8],
                            start=False, stop=True)
                        fsb = rwp.tile([128, 512], BF, tag="fsb", bufs=2,
                                       name=f"fsb_{rn}_{j}")
                        nc.scalar.activation(fsb[:, 0:w8], fps[:, 0:w8], AF.Sigmoid)
                        prod = rwp.tile([128, 512], BF, tag="fsb", bufs=2,
                                        name=f"prod_{rn}_{j}")
                        nc.vector.tensor_mul(prod[:, 0:w8], fsb[:, 0:w8],
                                             get_chH(j)[:, 0:w8])
                        group8_sum(prod[:, 0:w8], fcT[:, j, 0:nb], nb, rn, 100 + j)
                    # 6-7. gates + combine, all feature-major (no transposes)
                    si = rwp.tile([128, 8, NB[rn]], BF, tag="g", bufs=4,
                                  name=f"si_{rn}")
                    tu = rwp.tile([128, 8, NB[rn]], BF, tag="g", bufs=4,
                                  name=f"tu_{rn}")
                    so = rwp.tile([128, 8, NB[rn]], BF, tag="g", bufs=4,
                                  name=f"so_{rn}")
                    nc.scalar.activation(si[:, :, 0:nb], gps[0][:, :, 0:nb],
                                         AF.Sigmoid)
                    nc.scalar.activation(so[:, :, 0:nb], gps[1][:, :, 0:nb],
                                         AF.Sigmoid)
                    nc.scalar.activation(tu[:, :, 0:nb], gps[2][:, :, 0:nb],
                                         AF.Tanh)
                    p1 = rwp.tile([128, 8, NB[rn]], BF, tag="g", bufs=4,
                                  name=f"p1_{rn}")
                    nc.vector.tensor_mul(p1[:, :, 0:nb], si[:, :, 0:nb],
                                         tu[:, :, 0:nb])
                    cm = out_c()
                    nc.vector.tensor_add(cm, p1[:, :, 0:nb], fcT[:, :, 0:nb])
                    tanc = rwp.tile([128, 8, NB[rn]], BF, tag="tanc", bufs=2,
                                    name=f"tanc_{rn}")
                    nc.scalar.activation(tanc[:, :, 0:nb], cm, AF.Tanh)
                    nc.vector.tensor_mul(out_h(), so[:, :, 0:nb],
                                         tanc[:, :, 0:nb])

                def hc_ch(state, c0, w8):
                    return lambda m: hc[:, state, m, c0:c0 + w8]

                # ---- R4 ----
                run_round(0, 56, 0,
                          hc_ch(0, 0, 448), hc_ch(1, 0, 448),
                          lambda: st4[:, 0, :, 0:56],
                          lambda: st4[:, 1, :, 0:56])
                nc.sync.dma_start(ag_in[:], st4[:])
                nc.gpsimd.collective_compute(
                    "AllGather", mybir.AluOpType.bypass, replica_groups=RG,
                    ins=[ag_in.opt()], outs=[ag_out.opt()])
                # ---- R3a: last 9 parents (leaf-only children on every core)
                # overlaps with the AllGather ----
                run_round(1, 9, 56 + 55,
                          (lambda m: hcB[:, 0, m, 0:72]),
                          (lambda m: hcB[:, 1, m, 0:72]),
                          lambda: c3[:, 0, :, 55:64],
                          lambda: c3[:, 1, :, 55:64])
                pid = nc.gpsimd.partition_id()
                for r in range(NCORES):
                    off = 448 + (S4[r] - 585)
                    nc.gpsimd.dma_start(
                        hc[:, :, :, off:off + C4[r]],
                        ag_out[r, :, :, :, 0:C4[r]],
                        cond=(pid == 0))
                # ---- R3b: first 55 parents (needs gathered R4 on core 0) ----
                run_round(1, 55, 56,
                          hc_ch(0, 448, 440), hc_ch(1, 448, 440),
                          lambda: c3[:, 0, :, 0:55],
                          lambda: c3[:, 1, :, 0:55])
                # ---- R2 ----
                run_round(2, 8, 120,
                          lambda m: c3[:, 0, m, :], lambda m: c3[:, 1, m, :],
                          lambda: c2[:, 0, :, 0:8],
                          lambda: c2[:, 1, :, 0:8])
                # ---- R1 ----
                run_round(3, 1, 128,
                          lambda m: c2[:, 0, m, :], lambda m: c2[:, 1, m, :],
                          lambda: st1[:, 0, :, 0:1],
                          lambda: st1[:, 1, :, 0:1])
                nc.sync.dma_start(agb_in[:], st1[:])
                nc.gpsimd.collective_compute(
                    "AllGather", mybir.AluOpType.bypass, replica_groups=RG,
                    ins=[agb_in.opt()], outs=[agb_out.opt()])
                for r in range(NCORES):
                    nc.sync.dma_start(c1[:, :, :, r:r + 1], agb_out[r])
                # ---- R0 ----
                run_round(4, 1, 129,
                          lambda m: c1[:, 0, m, :], lambda m: c1[:, 1, m, :],
                          lambda: rootc_sb[:, :].rearrange("p (m n) -> p m n", n=1),
                          lambda: rooth_sb[:, :].rearrange("p (m n) -> p m n", n=1))
                nc.sync.dma_start(
                    rc_d[0, :].rearrange("(m p) -> p m", p=128), rootc_sb[:])
                nc.sync.dma_start(
                    rh_d[0, :].rearrange("(m p) -> p m", p=128), rooth_sb[:])

    nc.compile()
    return nc



def _preprocess(inputs, children, w_ioux, b_ioux, w_iouh, b_iouh,
                w_fx, b_fx, w_fh, b_fh):
    f32 = np.float32
    bf16 = DT.np(DT.bfloat16)
    inputs = np.ascontiguousarray(inputs, dtype=f32)
    b_tot = (np.asarray(b_ioux) + np.asarray(b_iouh)).astype(f32)
    b_fhx = (np.asarray(b_fx) + np.asarray(b_fh)).astype(f32)

    X = inputs.T                                           # [1024, 8192]
    Wcat = np.concatenate([np.asarray(w_ioux, dtype=f32),
                           np.asarray(w_fx, dtype=f32)], axis=0)   # [4096, 1024]
    WA = np.ascontiguousarray(Wcat.T.reshape(8, 128, 4096)).astype(bf16)
    WRcat = np.concatenate([np.asarray(w_iouh, dtype=f32).T,
                            np.asarray(w_fh, dtype=f32).T], axis=1)  # [1024, 4096]
    WR = np.ascontiguousarray(WRcat.reshape(8, 128, 4096)).astype(bf16)
    BT = np.empty((128, 32), dtype=f32)
    BT[:, 0:24] = b_tot.reshape(24, 128).T
    BT[:, 24:32] = b_fhx.reshape(8, 128).T
    SEL = np.zeros((128, 1024), dtype=f32)
    q = np.arange(1024)
    SEL[q // 8, q] = 1.0
    SEL = SEL.astype(bf16)
    I128 = np.eye(128, dtype=f32).astype(bf16)

    in_maps = []
    for i in range(NCORES):
        cols = _core_cols(i)
        mask = np.array([c >= 0 for c in cols])
        idx = np.array([max(c, 0) for c in cols])
        Xc = np.where(mask[None, :], X[:, idx], f32(0.0))   # [1024, 1096]
        xT = np.ascontiguousarray(Xc.reshape(8, 128, NCOLS)).astype(bf16)
        in_maps.append(dict(xT=xT, WA=WA, WR=WR, BT=BT, SEL=SEL, I128=I128))
    return in_maps


def kernel(**inputs):
    nc = _build()
    in_maps = _preprocess(**inputs)
    res = run_bass_kernel_spmd(nc, in_maps, list(range(NCORES))).results
    root_h = np.asarray(res[0]["root_h"], dtype=np.float32)
    root_c = np.asarray(res[0]["root_c"], dtype=np.float32)
    return root_h, root_c
